# revision 41
# baseline (speedup 1.0000x reference)
"""LIF layer (T=64, B=128, 2048->2048) on 8 trn2 NeuronCores.

Sharding: 4-way over out_dim x 2-way over batch. Core (g, h) owns
out channels [g*512, (g+1)*512) and batch rows [h*64, (h+1)*64).

Per core:
  GEMM  cur[o, (t,b)] = sum_i W[o,i] * x[t,b,i] as a single f32r pass
        (fp22-ish precision, 1 cyc/row, measured ~0.015 rel on spikes)
  SCAN  64 sequential LIF steps on [128, 4, 64] state tiles (DVE),
        reading cur straight out of PSUM.
Bias is folded away via the change of variable u = mem - b/(1-decay),
turning the per-step bias add into a per-channel spike threshold
(THR=1 so the reset subtract is just u -= spk).

Blocks of 512 columns (8 timesteps): PSUM tile [128, 4ot, 512] = 4
banks, double-buffered. Matmuls run ot-outer/kt-inner so each bank
sees a 16-matmul accumulation run (the fused f32r weight load then
hides almost fully behind the previous matmul's 512-col stream).

Prologue discipline (HBM is shared round-robin per DMA queue, SWDGE
descriptor gen is ~6 ns per partition-line): W and block-0 x are
interleaved on one queue in exact consumption order, block 0 runs
kt-outer to eat chunks as they land, and junk matmuls bridge the
supply-limited stretches so the PE's HAM clock stays at 2.4 GHz.

Host-side prep: transpose/pack x per batch half ([128, blk, kt, 512]
so each block DMA is one contiguous 3D transfer), slice/pack W,
precompute threshold/init tiles; output spikes return as bf16 and are
reassembled/cast on the host.
"""

import math

import numpy as np

import concourse.bacc as bacc
import concourse.mybir as mybir
import concourse.tile as tile
from concourse import bass_utils

# Problem constants (hardcoded per contract)
T, B, I, O = 64, 128, 2048, 2048
N_CORES = 8
GO, GB = 4, 2              # out-groups x batch-groups
OL = O // GO               # 512 out-channels per core
OT = OL // 128             # 4 out tiles
BL = B // GB               # 64 batch rows per core
COLS = T * BL              # 4096 (t,b) columns per core
KT = I // 128              # 16 k-tiles
NBLK = 8                   # col-blocks per core
BLK = COLS // NBLK         # 512 cols = 8 timesteps per block
TBLK = BLK // BL           # 8 timesteps per block
TAU, THR = 2.0, 1.0
DECAY = math.exp(-1.0 / TAU)

F32 = mybir.dt.float32
F32R = mybir.dt.float32r
BF16 = mybir.dt.bfloat16
ALU = mybir.AluOpType

MODE = "f32r-tp4dp2"

_cache = {}


def _build_nc():
    nc = bacc.Bacc(trn_type="TRN2", target_bir_lowering=False)

    xT_d = nc.dram_tensor("xT", [128, NBLK, KT, BLK], F32R, kind="ExternalInput")
    jnk_d = nc.dram_tensor("jnk", [128, 128], F32R, kind="ExternalInput")
    w_d = nc.dram_tensor("w", [128, KT, OT, 128], F32R, kind="ExternalInput")
    thr_d = nc.dram_tensor("thr", [128, OT, BL], F32, kind="ExternalInput")
    u0_d = nc.dram_tensor("u0", [128, OT, BL], F32, kind="ExternalInput")
    out_d = nc.dram_tensor("out", [128, T, OT, BL], BF16, kind="ExternalOutput")

    with tile.TileContext(nc) as tc:
        with (
            tc.tile_pool(name="wpool", bufs=1) as wpool,
            tc.tile_pool(name="xpool", bufs=9) as xpool,
            tc.tile_pool(name="state", bufs=1) as state,
            tc.tile_pool(name="spkpool", bufs=4) as spkpool,
            tc.tile_pool(name="psum", bufs=2, space="PSUM") as psum_pool,
        ):
            # Persistent state tiles (tiny, load first)
            u = state.tile([128, OT, BL], F32)
            thr_t = state.tile([128, OT, BL], F32)
            nc.gpsimd.dma_start(u[:], u0_d[:])
            nc.gpsimd.dma_start(thr_t[:], thr_d[:])

            # Warm the PE (HAM ramp) with junk matmuls while the first
            # real DMAs land. They write the warmup psum slot; every real
            # bank gets start=True on its kt==0 matmul, so values are safe.
            junk = state.tile([128, 128], F32R)
            nc.sync.dma_start(junk[:], jnk_d[:])
            ps_w = psum_pool.tile([128, OT, BLK], F32, tag="ps",
                                  name="ps_warm")
            for _ in range(14):
                nc.tensor.matmul(ps_w[:, 0, :128], junk[:], junk[:],
                                 start=True, stop=True)

            # Weights as separate chunk tiles so matmuls gate only on the
            # chunk they read (tile-granular deps): kt ranges per W_BOUNDS.
            # W and block-0 x are interleaved on ONE queue in the exact
            # order block 0's kt-outer matmuls consume them (HBM bandwidth
            # is shared round-robin per queue, so a dedicated W queue
            # starves the critical path).
            w_chunks = []

            def w_tile(kt):
                for lo, wc in reversed(w_chunks):
                    if kt >= lo:
                        return wc[:, kt - lo]
                raise AssertionError

            for bi in range(NBLK):
                # x for this block: kt-chunked DMAs (block 0 staircased and
                # interleaved with the W chunks, in consumption order)
                x_bounds = [0, 2, 4, 8, 12, KT] if bi == 0 else [0, 8, KT]
                xts = []
                for xi in range(len(x_bounds) - 1):
                    lo, hi = x_bounds[xi], x_bounds[xi + 1]
                    if bi == 0:
                        wc = wpool.tile([128, hi - lo, OT, 128], F32R,
                                        name=f"w_{xi}")
                        nc.sync.dma_start(wc[:], w_d[:, lo:hi])
                        w_chunks.append((lo, wc))
                    xt = xpool.tile([128, KT // 2, BLK], F32R, tag="xt",
                                    name=f"xt_{bi}_{xi}")
                    nc.sync.dma_start(xt[:, :hi - lo], xT_d[:, bi, lo:hi])
                    xts.append((lo, hi, xt))

                def x_slice(kt):
                    for lo, hi, xt in xts:
                        if lo <= kt < hi:
                            return xt[:, kt - lo, :]
                    raise AssertionError

                ps = psum_pool.tile([128, OT, BLK], F32, tag="ps",
                                    name=f"ps_{bi}")
                # Block 0 runs kt-outer so matmuls start as soon as each
                # (W, x) chunk pair lands; later blocks run ot-outer for
                # long same-bank accumulation runs.
                if bi == 0:
                    order = [(ot, kt) for kt in range(KT) for ot in range(OT)]
                else:
                    order = [(ot, kt) for ot in range(OT) for kt in range(KT)]
                junk_after = {1, 3, 7, 11} if bi == 0 else set()
                for ot, kt in order:
                    nc.tensor.matmul(
                        ps[:, ot, :],
                        w_tile(kt)[:, ot, :],
                        x_slice(kt),
                        start=(kt == 0),
                        stop=(kt == KT - 1),
                    )
                    # Bridge supply-limited chunk boundaries with junk
                    # matmuls so the HAM activity window stays warm.
                    if ot == OT - 1 and kt in junk_after:
                        for _ in range(4):
                            nc.tensor.matmul(ps_w[:, 0, :128], junk[:],
                                             junk[:], start=True, stop=True)

                # LIF steps consuming this block's PSUM; spikes land in two
                # half-block buffers so the out-DMA overlaps the scan.
                for hf in range(2):
                    spkb = spkpool.tile([128, TBLK // 2, OT, BL], BF16,
                                        tag="spk", name=f"spk_{bi}_{hf}")
                    for tj in range(TBLK // 2):
                        tl = hf * (TBLK // 2) + tj
                        nc.vector.scalar_tensor_tensor(
                            u[:], u[:], DECAY,
                            ps[:, :, tl * BL:(tl + 1) * BL],
                            op0=ALU.mult, op1=ALU.add)
                        nc.vector.tensor_tensor(
                            spkb[:, tj], u[:], thr_t[:], op=ALU.is_gt)
                        nc.vector.tensor_tensor(
                            u[:], u[:], spkb[:, tj], op=ALU.subtract)
                    t0 = bi * TBLK + hf * (TBLK // 2)
                    nc.gpsimd.dma_start(
                        out_d[:, t0:t0 + TBLK // 2], spkb[:])

    nc.compile()
    return nc


def _get_nc():
    if "nc" not in _cache:
        _cache["nc"] = _build_nc()
    return _cache["nc"]


def kernel(x_seq: np.ndarray, W: np.ndarray, b: np.ndarray) -> np.ndarray:
    nc = _get_nc()

    # Two distinct x shards (one per batch half), shared by 4 cores each.
    # Packed [128(p), NBLK, KT, BLK] so block DMAs are contiguous per
    # partition (few DMA descriptors).
    xTs = []
    for h in range(GB):
        xs = np.ascontiguousarray(
            x_seq[:, h * BL:(h + 1) * BL, :], dtype=np.float32)
        xT = xs.reshape(T * BL, I).T  # [I, COLS]
        xTs.append(np.ascontiguousarray(
            xT.reshape(KT, 128, NBLK, BLK).transpose(1, 2, 0, 3)))

    in_maps = []
    for c in range(N_CORES):
        g, h = c // GB, c % GB
        w_c = W[g * OL:(g + 1) * OL, :].astype(np.float32)      # [OL, I]
        wTc = np.ascontiguousarray(w_c.T)                       # [I, OL]
        wp = np.ascontiguousarray(
            wTc.reshape(KT, 128, OT, 128).transpose(1, 0, 2, 3))
        b_c = b[g * OL:(g + 1) * OL].astype(np.float32)         # [OL]
        shift = b_c / (1.0 - DECAY)
        thr = (THR - shift).reshape(OT, 128).T                  # [128, OT]
        u0 = (-shift).reshape(OT, 128).T
        thr_tile = np.ascontiguousarray(
            np.broadcast_to(thr[:, :, None], (128, OT, BL)), dtype=np.float32)
        u0_tile = np.ascontiguousarray(
            np.broadcast_to(u0[:, :, None], (128, OT, BL)), dtype=np.float32)
        in_maps.append({
            "xT": xTs[h], "w": wp, "thr": thr_tile, "u0": u0_tile,
            "jnk": np.zeros((128, 128), dtype=np.float32),
        })

    res = bass_utils.run_bass_kernel_spmd(nc, in_maps, core_ids=list(range(N_CORES)))
    global LAST_RESULT
    LAST_RESULT = res

    # Assemble: out_c[op, ot, t, b] -> [t, b, ot*128+op] per core block
    out = np.empty((T, B, O), dtype=np.float32)
    for c in range(N_CORES):
        g, h = c // GB, c % GB
        oc = res.results[c]["out"].astype(np.float32)  # [128, T, OT, BL]
        out[:, h * BL:(h + 1) * BL, g * OL:(g + 1) * OL] = (
            oc.transpose(1, 3, 2, 0).reshape(T, BL, OL))
    return out


LAST_RESULT = None


# revision 42
# speedup vs baseline: 1.0469x; 1.0469x over previous
"""LIF layer (T=64, B=128, 2048->2048) on 8 trn2 NeuronCores.

Sharding: 4-way over out_dim x 2-way over batch. Core (g, h) owns
out channels [g*512, (g+1)*512) and batch rows [h*64, (h+1)*64).

Per core:
  GEMM  cur[o, (t,b)] = sum_i W[o,i] * x[t,b,i] as a single f32r pass
        (fp22-ish precision, 1 cyc/row, measured ~0.015 rel on spikes)
  SCAN  64 sequential LIF steps on [128, 4, 64] state tiles (DVE),
        reading cur straight out of PSUM.
Bias is folded away via the change of variable u = mem - b/(1-decay),
turning the per-step bias add into a per-channel spike threshold
(THR=1 so the reset subtract is just u -= spk).

Blocks of 512 columns (8 timesteps): PSUM tile [128, 4ot, 512] = 4
banks, double-buffered. Matmuls run ot-outer/kt-inner so each bank
sees a 16-matmul accumulation run (the fused f32r weight load then
hides almost fully behind the previous matmul's 512-col stream).

Prologue discipline (HBM is shared round-robin per DMA queue, SWDGE
descriptor gen is ~6 ns per partition-line): W and block-0 x are
interleaved on one queue in exact consumption order, block 0 runs
kt-outer to eat chunks as they land, and junk matmuls bridge the
supply-limited stretches so the PE's HAM clock stays at 2.4 GHz.

Host-side prep: transpose/pack x per batch half ([128, blk, kt, 512]
so each block DMA is one contiguous 3D transfer), slice/pack W,
precompute threshold/init tiles; output spikes return as bf16 and are
reassembled/cast on the host.
"""

import math

import numpy as np

import concourse.bacc as bacc
import concourse.mybir as mybir
import concourse.tile as tile
from concourse import bass_utils

# Problem constants (hardcoded per contract)
T, B, I, O = 64, 128, 2048, 2048
N_CORES = 8
GO, GB = 4, 2              # out-groups x batch-groups
OL = O // GO               # 512 out-channels per core
OT = OL // 128             # 4 out tiles
BL = B // GB               # 64 batch rows per core
COLS = T * BL              # 4096 (t,b) columns per core
KT = I // 128              # 16 k-tiles
NBLK = 8                   # col-blocks per core
BLK = COLS // NBLK         # 512 cols = 8 timesteps per block
TBLK = BLK // BL           # 8 timesteps per block
TAU, THR = 2.0, 1.0
DECAY = math.exp(-1.0 / TAU)

F32 = mybir.dt.float32
F32R = mybir.dt.float32r
BF16 = mybir.dt.bfloat16
ALU = mybir.AluOpType

MODE = "f32r-tp4dp2"

_cache = {}


def _build_nc():
    nc = bacc.Bacc(trn_type="TRN2", target_bir_lowering=False)

    xT_d = nc.dram_tensor("xT", [128, NBLK, KT, BLK], F32R, kind="ExternalInput")
    jnk_d = nc.dram_tensor("jnk", [128, 128], F32R, kind="ExternalInput")
    w_d = nc.dram_tensor("w", [128, KT, OT, 128], F32R, kind="ExternalInput")
    thr_d = nc.dram_tensor("thr", [128, OT, BL], F32, kind="ExternalInput")
    u0_d = nc.dram_tensor("u0", [128, OT, BL], F32, kind="ExternalInput")
    out_d = nc.dram_tensor("out", [128, T, OT, BL], BF16, kind="ExternalOutput")

    with tile.TileContext(nc) as tc:
        with (
            tc.tile_pool(name="wpool", bufs=1) as wpool,
            tc.tile_pool(name="xpool", bufs=9) as xpool,
            tc.tile_pool(name="state", bufs=1) as state,
            tc.tile_pool(name="spkpool", bufs=4) as spkpool,
            tc.tile_pool(name="psum", bufs=2, space="PSUM") as psum_pool,
        ):
            # Persistent state tiles (tiny, load first)
            u = state.tile([128, OT, BL], F32)
            thr_t = state.tile([128, OT, BL], F32)
            nc.gpsimd.dma_start(u[:], u0_d[:])
            nc.gpsimd.dma_start(thr_t[:], thr_d[:])

            # Warm the PE (HAM ramp) with junk matmuls while the first
            # real DMAs land. They write the warmup psum slot; every real
            # bank gets start=True on its kt==0 matmul, so values are safe.
            junk = state.tile([128, 128], F32R)
            nc.sync.dma_start(junk[:], jnk_d[:])
            ps_w = psum_pool.tile([128, OT, BLK], F32, tag="ps",
                                  name="ps_warm")
            for _ in range(14):
                nc.tensor.matmul(ps_w[:, 0, :128], junk[:], junk[:],
                                 start=True, stop=True)

            # Weights as separate chunk tiles so matmuls gate only on the
            # chunk they read (tile-granular deps): kt ranges per W_BOUNDS.
            # W and block-0 x are interleaved on ONE queue in the exact
            # order block 0's kt-outer matmuls consume them (HBM bandwidth
            # is shared round-robin per queue, so a dedicated W queue
            # starves the critical path).
            w_chunks = []

            def w_tile(kt):
                for lo, wc in reversed(w_chunks):
                    if kt >= lo:
                        return wc[:, kt - lo]
                raise AssertionError

            for bi in range(NBLK):
                # x for this block: kt-chunked DMAs (block 0 staircased and
                # interleaved with the W chunks, in consumption order)
                x_bounds = [0, 2, 4, 8, 12, KT] if bi == 0 else [0, 8, KT]
                xts = []
                for xi in range(len(x_bounds) - 1):
                    lo, hi = x_bounds[xi], x_bounds[xi + 1]
                    if bi == 0:
                        wc = wpool.tile([128, hi - lo, OT, 128], F32R,
                                        name=f"w_{xi}")
                        nc.sync.dma_start(wc[:], w_d[:, lo:hi])
                        w_chunks.append((lo, wc))
                    xt = xpool.tile([128, KT // 2, BLK], F32R, tag="xt",
                                    name=f"xt_{bi}_{xi}")
                    nc.sync.dma_start(xt[:, :hi - lo], xT_d[:, bi, lo:hi])
                    xts.append((lo, hi, xt))

                def x_slice(kt):
                    for lo, hi, xt in xts:
                        if lo <= kt < hi:
                            return xt[:, kt - lo, :]
                    raise AssertionError

                ps = psum_pool.tile([128, OT, BLK], F32, tag="ps",
                                    name=f"ps_{bi}")
                # Block 0 runs kt-outer so matmuls start as soon as each
                # (W, x) chunk pair lands; later blocks run ot-outer for
                # long same-bank accumulation runs.
                if bi == 0:
                    order = [(ot, kt) for kt in range(KT) for ot in range(OT)]
                else:
                    order = [(ot, kt) for ot in range(OT) for kt in range(KT)]
                junk_after = {1, 3, 7, 11} if bi == 0 else set()
                for ot, kt in order:
                    nc.tensor.matmul(
                        ps[:, ot, :],
                        w_tile(kt)[:, ot, :],
                        x_slice(kt),
                        start=(kt == 0),
                        stop=(kt == KT - 1),
                    )
                    # Bridge supply-limited chunk boundaries with junk
                    # matmuls so the HAM activity window stays warm.
                    if ot == OT - 1 and kt in junk_after:
                        for _ in range(4):
                            nc.tensor.matmul(ps_w[:, 0, :128], junk[:],
                                             junk[:], start=True, stop=True)

                # LIF steps consuming this block's PSUM; spikes land in two
                # half-block buffers so the out-DMA overlaps the scan. The
                # last block uses per-step buffers/DMAs so only one step's
                # transfer trails the final scan op, and skips the dead
                # final membrane update.
                last = bi == NBLK - 1
                steps_per_buf = 1 if last else TBLK // 2
                for hf in range(TBLK // steps_per_buf):
                    spkb = spkpool.tile([128, steps_per_buf, OT, BL], BF16,
                                        tag="spk1" if last else "spk",
                                        name=f"spk_{bi}_{hf}")
                    for tj in range(steps_per_buf):
                        tl = hf * steps_per_buf + tj
                        nc.vector.scalar_tensor_tensor(
                            u[:], u[:], DECAY,
                            ps[:, :, tl * BL:(tl + 1) * BL],
                            op0=ALU.mult, op1=ALU.add)
                        nc.vector.tensor_tensor(
                            spkb[:, tj], u[:], thr_t[:], op=ALU.is_gt)
                        if not (last and tl == TBLK - 1):
                            nc.vector.tensor_tensor(
                                u[:], u[:], spkb[:, tj], op=ALU.subtract)
                    t0 = bi * TBLK + hf * steps_per_buf
                    nc.gpsimd.dma_start(
                        out_d[:, t0:t0 + steps_per_buf], spkb[:])

    nc.compile()
    return nc


def _get_nc():
    if "nc" not in _cache:
        _cache["nc"] = _build_nc()
    return _cache["nc"]


def kernel(x_seq: np.ndarray, W: np.ndarray, b: np.ndarray) -> np.ndarray:
    nc = _get_nc()

    # Two distinct x shards (one per batch half), shared by 4 cores each.
    # Packed [128(p), NBLK, KT, BLK] so block DMAs are contiguous per
    # partition (few DMA descriptors).
    xTs = []
    for h in range(GB):
        xs = np.ascontiguousarray(
            x_seq[:, h * BL:(h + 1) * BL, :], dtype=np.float32)
        xT = xs.reshape(T * BL, I).T  # [I, COLS]
        xTs.append(np.ascontiguousarray(
            xT.reshape(KT, 128, NBLK, BLK).transpose(1, 2, 0, 3)))

    in_maps = []
    for c in range(N_CORES):
        g, h = c // GB, c % GB
        w_c = W[g * OL:(g + 1) * OL, :].astype(np.float32)      # [OL, I]
        wTc = np.ascontiguousarray(w_c.T)                       # [I, OL]
        wp = np.ascontiguousarray(
            wTc.reshape(KT, 128, OT, 128).transpose(1, 0, 2, 3))
        b_c = b[g * OL:(g + 1) * OL].astype(np.float32)         # [OL]
        shift = b_c / (1.0 - DECAY)
        thr = (THR - shift).reshape(OT, 128).T                  # [128, OT]
        u0 = (-shift).reshape(OT, 128).T
        thr_tile = np.ascontiguousarray(
            np.broadcast_to(thr[:, :, None], (128, OT, BL)), dtype=np.float32)
        u0_tile = np.ascontiguousarray(
            np.broadcast_to(u0[:, :, None], (128, OT, BL)), dtype=np.float32)
        in_maps.append({
            "xT": xTs[h], "w": wp, "thr": thr_tile, "u0": u0_tile,
            "jnk": np.zeros((128, 128), dtype=np.float32),
        })

    res = bass_utils.run_bass_kernel_spmd(nc, in_maps, core_ids=list(range(N_CORES)))
    global LAST_RESULT
    LAST_RESULT = res

    # Assemble: out_c[op, ot, t, b] -> [t, b, ot*128+op] per core block
    out = np.empty((T, B, O), dtype=np.float32)
    for c in range(N_CORES):
        g, h = c // GB, c % GB
        oc = res.results[c]["out"].astype(np.float32)  # [128, T, OT, BL]
        out[:, h * BL:(h + 1) * BL, g * OL:(g + 1) * OL] = (
            oc.transpose(1, 3, 2, 0).reshape(T, BL, OL))
    return out


LAST_RESULT = None
